# revision 7
# baseline (speedup 1.0000x reference)
"""GNN message-passing kernel for Trainium2 (8 NeuronCores, batch-parallel).

Computation (per reference):
    norm_adj = adjacency * dinv * dinv.T + I            [10,10]   (host, O(100) flops)
    support  = einsum('bcf,fo->bco', x, kernel)         [B,C,O]
    out      = elu(einsum('ij,bjo->bio', norm_adj, support) + bias)
    out      = (out - mean) * rsqrt(var+eps) * gamma + beta

Device strategy per core (512 batches = 5120 rows of [b,c] x f):
  1. "Transposing mix": PE matmul with x-chunks [crows<=120, 128f] as the
     stationary operand and a block-diagonal norm_adj matrix [crows, 256pad]
     as the moving operand. One op both applies the channel mix and lands
     the activations transposed ([f, rows]) as needed by the main matmul.
  2. Main matmul, fp32r full-rate: outT[o,rows] += K[f,o].T @ yT[f,rows],
     kernel matrix resident in SBUF.
  3. Epilogue on ACT/DVE with per-partition (o) params:
     elu(z) = min(exp(z), z+1) - 1 (exact), then folded BN affine.
     Output stored transposed [O, rows]; host transposes while unsharding.
"""

import os
from contextlib import ExitStack

import numpy as np

import concourse.bass as bass
import concourse.bacc as bacc
import concourse.mybir as mybir
import concourse.tile as tile
from concourse.bass_utils import run_bass_kernel_spmd

F32 = mybir.dt.float32
F32R = mybir.dt.float32r
ALU = mybir.AluOpType
ACTF = mybir.ActivationFunctionType

P = 128
BN_EPS = 1e-3
N_CORES = 8
C = 10  # channels


def build_nc(rows, F, O, chunk_batches=(12, 12, 8), n_cores=N_CORES):
    """Build the per-core Bass program. rows = local (b,c) rows, F/O = feat dims."""
    panel = sum(chunk_batches) * C  # rows per panel (multiple of 10, >=256)
    assert rows % panel == 0
    n_panels = rows // panel
    FT, OT = F // P, O // P
    bd_sizes = sorted({nb * C for nb in chunk_batches})

    nc = bacc.Bacc(
        "TRN2",
        target_bir_lowering=False,
        debug=False,
        enable_asserts=False,
        num_devices=n_cores,
    )
    x_d = nc.dram_tensor("x_local", [rows, F], F32, kind="ExternalInput").ap()
    k_d = nc.dram_tensor("kern", [F, O], F32, kind="ExternalInput").ap()
    bd_d = {
        sz: nc.dram_tensor(f"bd{sz}", [sz, 256], F32, kind="ExternalInput").ap()
        for sz in bd_sizes
    }
    # prm columns: [0:OT]=bias_t, [OT:2OT]=scale_t, [2OT:3OT]=shift2_t  (per-partition o)
    prm_d = nc.dram_tensor("prm", [P, 3 * OT], F32, kind="ExternalInput").ap()
    outT_d = nc.dram_tensor("outT", [O, rows], F32, kind="ExternalOutput").ap()

    with tile.TileContext(nc) as tc, ExitStack() as ctx:
        const = ctx.enter_context(tc.tile_pool(name="const", bufs=1))
        kb = []
        for fb in range(FT):
            t = const.tile([P, O], F32R, name=f"kb{fb}", tag=f"kb{fb}")
            nc.sync.dma_start(t, k_d[fb * P : (fb + 1) * P, :].bitcast(F32R))
            kb.append(t)
        bd_t = {}
        for sz in bd_sizes:
            bt = const.tile([sz, 256], F32R, name=f"bd{sz}", tag=f"bd{sz}")
            nc.sync.dma_start(bt, bd_d[sz].bitcast(F32R))
            bd_t[sz] = bt
        prm = const.tile([P, 3 * OT], F32, name="prm")
        nc.sync.dma_start(prm, prm_d)

        xpool = ctx.enter_context(tc.tile_pool(name="xpool", bufs=3))
        ypool = ctx.enter_context(tc.tile_pool(name="ypool", bufs=1))
        mixps = ctx.enter_context(tc.tile_pool(name="mixps", bufs=2, space="PSUM"))
        mainps = ctx.enter_context(tc.tile_pool(name="mainps", bufs=4, space="PSUM"))
        tmp = ctx.enter_context(tc.tile_pool(name="tmp", bufs=2))

        for pi in range(n_panels):
            row0 = pi * panel
            ytall = ypool.tile([P, FT, panel], F32R, name=f"yt_{pi}", tag="yt")
            # ---- mix phase: yT[f, rows_panel] = blockdiag(normadj) applied to x
            coff = 0
            for ci, nb in enumerate(chunk_batches):
                crows = nb * C
                xt = xpool.tile([120, F], F32R, name=f"x_{pi}_{ci}", tag="xc")[:crows]
                nc.sync.dma_start(xt, x_d[row0 + coff : row0 + coff + crows, :].bitcast(F32R))
                for fbp in range(FT // 2):
                    fb = 2 * fbp
                    ps = mixps.tile([P, 2, 256], F32, name=f"mps_{pi}_{ci}_{fbp}", tag="mixps")
                    for half in range(2):
                        nc.tensor.matmul(
                            ps[:, half, :],
                            lhsT=xt[:, (fb + half) * P : (fb + half + 1) * P],
                            rhs=bd_t[crows],
                            start=True,
                            stop=True,
                        )
                    nc.vector.tensor_copy(
                        ytall[:, fb : fb + 2, coff : coff + crows], ps[:, :, :crows]
                    )
                coff += crows
            # ---- main matmul + epilogue per o-tile
            for ot in range(OT):
                ps = mainps.tile([P, panel], F32, name=f"ops_{pi}_{ot}", tag="mainps")
                for fb in range(FT):
                    nc.tensor.matmul(
                        ps,
                        lhsT=kb[fb][:, ot * P : (ot + 1) * P],
                        rhs=ytall[:, fb, :],
                        start=(fb == 0),
                        stop=(fb == FT - 1),
                    )
                bias_ap = prm[:, ot : ot + 1]
                scale_ap = prm[:, OT + ot : OT + ot + 1]
                shift_ap = prm[:, 2 * OT + ot : 2 * OT + ot + 1]
                e = tmp.tile([P, panel], F32, name=f"e_{pi}_{ot}", tag="e")
                t0 = tmp.tile([P, panel], F32, name=f"t0_{pi}_{ot}", tag="t0")
                s = tmp.tile([P, panel], F32, name=f"s_{pi}_{ot}", tag="s")
                fin = tmp.tile([P, panel], F32, name=f"fin_{pi}_{ot}", tag="fin")
                nc.scalar.activation(e, ps, ACTF.Exp, bias=bias_ap)
                nc.scalar.activation(t0, ps, ACTF.Relu, bias=bias_ap)
                # elu(zb) + 1 = min(exp(zb), relu(zb) + 1)   (exact identity)
                nc.vector.scalar_tensor_tensor(
                    s, in0=t0, scalar=1.0, in1=e, op0=ALU.add, op1=ALU.min
                )
                # fin = s*scale + (shift - scale) = elu*scale + shift
                nc.vector.tensor_scalar(
                    fin, s, scale_ap, shift_ap, op0=ALU.mult, op1=ALU.add
                )
                nc.sync.dma_start(outT_d[ot * P : (ot + 1) * P, row0 : row0 + panel], fin)
    nc.compile()
    return nc


def _host_prep(adjacency, kern, bias, gamma, beta, moving_mean, moving_var,
               chunk_batches=(12, 12, 8), O=2048):
    """Build the tiny derived inputs on the host."""
    A = np.asarray(adjacency, np.float32)
    deg = np.maximum(np.abs(A).sum(axis=1, keepdims=True), 1e-8)
    dinv = deg ** -0.5
    na = A * dinv * dinv.T + np.eye(C, dtype=np.float32)  # [10,10]

    bds = {}
    for nb in sorted(set(chunk_batches)):
        sz = nb * C
        bd = np.zeros((sz, 256), np.float32)
        for g in range(nb):
            bd[g * C : (g + 1) * C, g * C : (g + 1) * C] = na.T
        bds[sz] = bd

    OT = O // P
    scale = np.asarray(gamma, np.float32) / np.sqrt(np.asarray(moving_var, np.float32) + BN_EPS)
    shift2 = np.asarray(beta, np.float32) - np.asarray(moving_mean, np.float32) * scale - scale
    prm = np.empty((P, 3 * OT), np.float32)
    prm[:, 0:OT] = np.asarray(bias, np.float32).reshape(OT, P).T
    prm[:, OT : 2 * OT] = scale.reshape(OT, P).T
    prm[:, 2 * OT : 3 * OT] = shift2.reshape(OT, P).T
    return bds, prm


def kernel(x, adjacency, kernel, bias, gamma, beta, moving_mean, moving_var):
    B, C_, F = x.shape
    O = kernel.shape[1]
    assert C_ == C
    assert B % N_CORES == 0
    bl = B // N_CORES
    rows = bl * C

    chunk_batches = (12, 12, 8)
    bds, prm = _host_prep(adjacency, kernel, bias, gamma, beta, moving_mean,
                          moving_var, chunk_batches, O)

    nc = build_nc(rows, F, O, chunk_batches)

    kern_np = np.ascontiguousarray(np.asarray(kernel, np.float32))
    x_np = np.asarray(x, np.float32)
    in_maps = []
    for c in range(N_CORES):
        m = {
            "x_local": np.ascontiguousarray(x_np[c * bl : (c + 1) * bl].reshape(rows, F)),
            "kern": kern_np,
            "prm": prm,
        }
        for sz, bd in bds.items():
            m[f"bd{sz}"] = bd
        in_maps.append(m)

    res = run_bass_kernel_spmd(nc, in_maps, core_ids=list(range(N_CORES)), trace=False)

    out = np.empty((B, C, O), np.float32)
    for c in range(N_CORES):
        outT = res.results[c]["outT"]  # [O, rows]
        out[c * bl : (c + 1) * bl] = outT.T.reshape(bl, C, O)
    return out
